# revision 35
# baseline (speedup 1.0000x reference)
"""Distributed Trainium2 Bass kernel for the AttentionBlock problem.

Math (per batch b):
  q/k/v = x @ W + b ; scores = (q.k^T)/8 + pos[b,k,h], masked -> -inf,
  dummy col 0 ; pattern = softmax ; out = LayerNorm((pattern @ v) @ W_O)

Strategy (8 cores = 2 batches x 4 head-groups of 4 heads):
  * Host-side key compaction: masked keys are removed; key axis 2048 ->
    ~1046, padded to skp (mult of 128, always >= 1 pad).  Pad keys carry
    m=0 so they are exactly inert.  The LAST pad key is the softmax
    dummy: host solves W_K^T c = -b_K (lstsq) for its x-column so its
    projected k is ~0 -> score ~0 = DUMMY_SCORE, and m=1 -> it adds
    exactly +1 to the denominator via the 65th V column.  No device-side
    masking, biasing, or +1 ops anywhere.
  * Multiplicative softmax rewrite: exp(q.k/8 + pos) = exp(q.k/8)*m,
    m = exp(pos) host-computed; m scales V rows and the denominator
    column.  1/8 folded into W_Q/b_Q host-side.
  * All inputs are host-pre-swizzled into the exact SBUF byte layout so
    every load is a contiguous-run DMA (~line rate); loads are split
    across the sync/scalar/gpsimd queues and interleaved so K-proj ->
    first scores start within a few us.
  * Scores: two heads of a pair packed into one PE pass via row tiling
    (K=64 each, concurrent) into a 2-bank PSUM tile; one ACT exp call
    covers both heads (N=1024).  Scores are emitted one kt ahead of the
    z matmuls so the PE never waits on the ACT exp (ACT is the phase-2
    bottleneck at ~1us/kt).
  * z: per-head matmul with a 65th column of m in V, accumulating the
    softmax denominator for free.  bias b_V enters via
    z = (z_raw - b_V)/d + b_V.
  * Per 512-row block, per head-pair: a half-size 8-core AllToAll
    exchanges z^T (dup per batch group, receiver selects its batch via
    input-driven 0/1 scalars) so every core out-projects only its own
    128 rows.  Pair-level splitting starts the serialized cc stream
    mid-block and halves the last transfer.  A tiny warmup AllToAll is
    the first instruction of the kernel: it absorbs the ~70us cold-
    stream penalty + inter-core skew that otherwise hits the first real
    exchange.  Tail (out-proj) for block rb runs during block rb+1's
    attention; LayerNorm is deferred to the end (single ACT table
    switch), finished with one ACT Identity(scale=rstd, bias=-mu*rstd)
    per block.
"""

import os
from contextlib import ExitStack

import numpy as np

import concourse.bass as bass
import concourse.tile as tile
from concourse import bacc, mybir
from concourse.bass_utils import run_bass_kernel_spmd

B, SQ = 2, 2048
D = 1024
H, HS = 16, 64
ED = 1024
NCORES = 8
GROUP = 4          # cores per batch
HPC = 4            # heads per core
NRB = 4            # 512-row blocks per batch
RBS = 512
NDT = D // 128

F32 = mybir.dt.float32
BF16 = mybir.dt.bfloat16
FP8 = mybir.dt.float8e4
AF = mybir.ActivationFunctionType
ALU = mybir.AluOpType

LN_EPS = 1e-5
KVER = 10   # bump on every kernel revision: pads mt's shape so the HLO
           # (and thus the NEFF compile-cache key) is unique per version

LAST_EXEC_NS = None
_CACHED = {}


def _build(skp, ln_trivial, debug=False):
    nkt = skp // 128
    kblocks = [(s, min(512, skp - s)) for s in range(0, skp, 512)]

    nc = bacc.Bacc(None, target_bir_lowering=False)

    xq = nc.dram_tensor("xq", [128, NRB, NDT, RBS], BF16, kind="ExternalInput")
    xk = nc.dram_tensor("xk", [128, NDT, skp], BF16, kind="ExternalInput")
    xv = nc.dram_tensor("xv", [128, NDT, skp], BF16, kind="ExternalInput")
    wq = nc.dram_tensor("wq", [128, NDT, HPC * HS], BF16, kind="ExternalInput")
    wk = nc.dram_tensor("wk", [128, NDT, HPC * HS], BF16, kind="ExternalInput")
    wv = nc.dram_tensor("wv", [128, NDT, HPC * HS], BF16, kind="ExternalInput")
    wo = nc.dram_tensor("wo", [128, NDT, ED], BF16, kind="ExternalInput")
    bq = nc.dram_tensor("bq", [128, 2], F32, kind="ExternalInput")
    bk = nc.dram_tensor("bk", [128, 2], F32, kind="ExternalInput")
    bvt = nc.dram_tensor("bvt", [64, HPC], F32, kind="ExternalInput")
    bsel = nc.dram_tensor("bsel", [128, 2], F32, kind="ExternalInput")
    mt = nc.dram_tensor("mt", [128, nkt * HPC + KVER], F32, kind="ExternalInput")
    if not ln_trivial:
        lng = nc.dram_tensor("lng", [1, ED], BF16, kind="ExternalInput")
        lnb = nc.dram_tensor("lnb", [1, ED], BF16, kind="ExternalInput")
    out = nc.dram_tensor("out", [NRB * 128, ED], F32, kind="ExternalOutput")
    if debug:
        dbg_kt = nc.dram_tensor("dbg_kt", [128, 2, skp], BF16,
                                kind="ExternalOutput")
        dbg_qa = nc.dram_tensor("dbg_qa", [128, 2, SQ], BF16,
                                kind="ExternalOutput")
        dbg_v = nc.dram_tensor("dbg_v", [128, nkt, HPC, 65], BF16,
                               kind="ExternalOutput")
        dbg_y = nc.dram_tensor("dbg_y", [128, NRB, ED], BF16,
                               kind="ExternalOutput")
        dbg_zn = nc.dram_tensor("dbg_zn", [128, HPC, 512], BF16,
                                kind="ExternalOutput")
        dbg_ao = nc.dram_tensor("dbg_ao", [1024, 128], BF16,
                                kind="ExternalOutput")

    with tile.TileContext(nc) as tc, ExitStack() as ctx:
        consts = ctx.enter_context(tc.tile_pool(name="consts", bufs=1))
        res = ctx.enter_context(tc.tile_pool(name="res", bufs=1))
        dram = ctx.enter_context(tc.tile_pool(name="dram", bufs=8, space="DRAM"))
        pss = ctx.enter_context(tc.tile_pool(name="pss", bufs=2, space="PSUM"))
        psz = ctx.enter_context(tc.tile_pool(name="psz", bufs=2, space="PSUM"))
        psp = ctx.enter_context(tc.tile_pool(name="psp", bufs=2, space="PSUM"))
        ptp = ctx.enter_context(tc.tile_pool(name="ptp", bufs=3))
        ev = ctx.enter_context(tc.tile_pool(name="ev", bufs=2))
        ztp = ctx.enter_context(tc.tile_pool(name="ztp", bufs=2))
        lnp = ctx.enter_context(tc.tile_pool(name="lnp", bufs=4))

        # ---- constants (scalar queue, tiny, first) ----
        bq_sb = consts.tile([128, 2], F32)
        nc.scalar.dma_start(out=bq_sb, in_=bq[:, :])
        bk_sb = consts.tile([128, 2], F32)
        nc.scalar.dma_start(out=bk_sb, in_=bk[:, :])
        m_sb = consts.tile([128, nkt, HPC], F32)
        nc.scalar.dma_start(out=m_sb, in_=mt[:, 0:nkt * HPC].rearrange(
            "p (t h) -> p t h", t=nkt))
        bvt_sb = consts.tile([64, HPC], F32)
        nc.scalar.dma_start(out=bvt_sb, in_=bvt[:, :])
        bsel_sb = consts.tile([128, 2], F32)
        nc.scalar.dma_start(out=bsel_sb, in_=bsel[:, :])
        eps_sb = consts.tile([128, 1], F32)
        nc.vector.memset(eps_sb, LN_EPS)
        ones_c = consts.tile([1, 64], BF16)
        nc.vector.memset(ones_c, 1.0)

        # ---- persistent tiles ----
        kT_res = res.tile([128, 2, skp], BF16)       # [hs(pair-packed), pair, key]
        qa_sb = res.tile([128, 2, SQ], BF16)         # [hs(pair-packed), pair, row]
        v_res = res.tile([128, nkt, HPC, 65], BF16)  # [key, kt, head, hs|m]
        wo_sb = res.tile([128, NDT, ED], BF16)
        ystage = res.tile([128, NRB, ED], BF16)
        xq_sb = res.tile([128, NRB, NDT, RBS], BF16)
        xk_sb = res.tile([128, NDT, skp], BF16)
        xv_sb = res.tile([128, NDT, skp], BF16)
        wq_sb = res.tile([128, NDT, HPC * HS], BF16)
        wk_sb = res.tile([128, NDT, HPC * HS], BF16)
        wv_sb = res.tile([128, NDT, HPC * HS], BF16)

        # ---- input DMAs ----
        # SWDGE (gpsimd) sprays across all 16 SDMA engines (~430 GB/s
        # measured) while HWDGE (sync/scalar) moves ~100-150 GB/s: the
        # whole critical path rides gpsimd, ordered so K-proj -> Q-proj
        # -> first scores unblock ASAP.  sync gets the late q blocks,
        # scalar the tail-only wo.
        # warmup collective FIRST (before any DMA): the first AllToAll on
        # a cold cc stream runs 3-10x slower and absorbs inter-core start
        # skew; firing it at t=0 takes both off the critical path
        cw_in = dram.tile([128, 128], BF16, tag="cw", name="cwi")
        cw_out = dram.tile([128, 128], BF16, tag="cw2", name="cwo")
        nc.gpsimd.collective_compute(
            "AllToAll",
            ALU.bypass,
            replica_groups=[[0, 1, 2, 3, 4, 5, 6, 7]],
            ins=[cw_in[:, :].opt()],
            outs=[cw_out[:, :].opt()],
        )
        nc.gpsimd.dma_start(out=wk_sb, in_=wk[:, :, :])
        ks0, kw0 = kblocks[0]
        nc.gpsimd.dma_start(
            out=xk_sb[:, :, ks0:ks0 + kw0], in_=xk[:, :, ks0:ks0 + kw0])
        nc.gpsimd.dma_start(out=wq_sb, in_=wq[:, :, :])
        nc.gpsimd.dma_start(out=xq_sb[:, 0], in_=xq[:, 0])
        nc.gpsimd.dma_start(out=wv_sb, in_=wv[:, :, :])
        nc.gpsimd.dma_start(out=xv_sb[:, :, 0:128], in_=xv[:, :, 0:128])
        for (ks, kw) in kblocks[1:]:
            nc.gpsimd.dma_start(
                out=xk_sb[:, :, ks:ks + kw], in_=xk[:, :, ks:ks + kw])
        for ks in range(128, skp, 512):
            kw = min(512, skp - ks)
            nc.gpsimd.dma_start(
                out=xv_sb[:, :, ks:ks + kw], in_=xv[:, :, ks:ks + kw])
        if not ln_trivial:
            g_bc = consts.tile([128, ED], BF16)
            b_bc = consts.tile([128, ED], BF16)
            nc.gpsimd.dma_start(out=g_bc, in_=lng[:, :].to_broadcast([128, ED]))
            nc.gpsimd.dma_start(out=b_bc, in_=lnb[:, :].to_broadcast([128, ED]))
        # sync: q block 1 then (later) a2a staging + zt loads
        nc.sync.dma_start(out=xq_sb[:, 1], in_=xq[:, 1])
        # scalar: late q blocks + wo (needed from rb2 / tail0 on)
        nc.scalar.dma_start(out=xq_sb[:, 2], in_=xq[:, 2])
        nc.scalar.dma_start(out=xq_sb[:, 3], in_=xq[:, 3])
        nc.scalar.dma_start(out=wo_sb, in_=wo[:, :, :])

        def emit_qproj(pair, qb):
            ps = psp.tile([128, 512], F32, tag="p", name=f"pq{pair}_{qb}")
            for dt in range(NDT):
                nc.tensor.matmul(
                    ps,
                    lhsT=wq_sb[:, dt, pair * 128:(pair + 1) * 128],
                    rhs=xq_sb[:, qb, dt, :],
                    start=(dt == 0), stop=(dt == NDT - 1),
                )
            nc.vector.tensor_scalar_add(
                out=qa_sb[:, pair, qb * RBS:(qb + 1) * RBS], in0=ps,
                scalar1=bq_sb[:, pair:pair + 1],
            )

        def emit_kproj(bi):
            ks, kw = kblocks[bi]
            for pair in range(2):
                ps = psp.tile([128, 512], F32, tag="p", name=f"pk{pair}_{ks}")
                for dt in range(NDT):
                    nc.tensor.matmul(
                        ps[:, 0:kw],
                        lhsT=wk_sb[:, dt, pair * 128:(pair + 1) * 128],
                        rhs=xk_sb[:, dt, ks:ks + kw],
                        start=(dt == 0), stop=(dt == NDT - 1),
                    )
                nc.vector.tensor_scalar_add(
                    out=kT_res[:, pair, ks:ks + kw], in0=ps[:, 0:kw],
                    scalar1=bk_sb[:, pair:pair + 1],
                )

        # ---- phase 1: K proj block 0 + first Q block; K blocks 1+ are
        # interleaved into rb0/pair0 (their kt tiles are consumed later)
        emit_kproj(0)
        emit_qproj(0, 0)
        emit_qproj(1, 0)

        def emit_vproj(kt):
            # V projection for one key tile, scaled by m; 65th col = m
            ps = psp.tile([128, 512], F32, tag="p", name=f"pv{kt}")
            for dt in range(NDT):
                nc.tensor.matmul(
                    ps[:, 0:HPC * HS],
                    lhsT=xv_sb[:, dt, kt * 128:(kt + 1) * 128],
                    rhs=wv_sb[:, dt, :],
                    start=(dt == 0), stop=(dt == NDT - 1),
                )
            for hl in range(HPC):
                nc.vector.tensor_scalar_mul(
                    out=v_res[:, kt, hl, 0:64],
                    in0=ps[:, hl * 64:(hl + 1) * 64],
                    scalar1=m_sb[:, kt, hl:hl + 1],
                )
            nc.vector.tensor_copy(out=v_res[:, kt, :, 64], in_=m_sb[:, kt, :])

        # ---- phase 2 ----
        a2a_outs = []
        a2a_halves = []
        mv_tiles = []
        SPLIT_RB = {0, NRB - 1}

        def emit_scores(rb, pair, kt):
            rs = slice(rb * RBS, (rb + 1) * RBS)
            s2 = pss.tile([128, 1024], F32, tag="s", name=f"s{rb}{pair}{kt}")
            nc.tensor.matmul(
                s2[:, 0:512],
                lhsT=kT_res[0:64, pair, kt * 128:(kt + 1) * 128],
                rhs=qa_sb[0:64, pair, rs],
                start=True, stop=True,
            )
            nc.tensor.matmul(
                s2[:, 512:1024],
                lhsT=kT_res[64:128, pair, kt * 128:(kt + 1) * 128],
                rhs=qa_sb[64:128, pair, rs],
                start=True, stop=True,
            )
            return s2

        def emit_tail(rb):
            # out-projection of this core's 128 rows for block rb; load
            # both batch halves, select mine via input-driven 0/1 scalar.
            # Split-rb form: dt tile (2j + p) <-> sel[p][:, j]; MMs run
            # pair0's dts first so they start before pair1's exchange
            # lands.  Full-rb form: one [128, 8, 128] zt, natural order.
            entry = a2a_outs[rb]
            split = isinstance(entry, tuple)
            if split:
                sel = []
                for p in range(2):
                    ao = entry[p]
                    zt0 = ztp.tile([128, 4, 128], BF16, tag=f"z0{p}",
                                   name=f"zt0_{rb}{p}")
                    nc.sync.dma_start(
                        out=zt0,
                        in_=ao[0:512, :].rearrange("(t q) r -> q t r", q=128))
                    zt1 = ztp.tile([128, 4, 128], BF16, tag=f"z1{p}",
                                   name=f"zt1_{rb}{p}")
                    nc.sync.dma_start(
                        out=zt1,
                        in_=ao[512:1024, :].rearrange("(t q) r -> q t r", q=128))
                    zt = ztp.tile([128, 4, 128], BF16, tag=f"zs{p}",
                                  name=f"zt{rb}{p}")
                    nc.vector.tensor_scalar_mul(
                        out=zt, in0=zt1, scalar1=bsel_sb[:, 1:2])
                    nc.vector.scalar_tensor_tensor(
                        out=zt, in0=zt0, scalar=bsel_sb[:, 0:1], in1=zt,
                        op0=ALU.mult, op1=ALU.add,
                    )
                    sel.append(zt)
                dt_seq = [0, 2, 4, 6, 1, 3, 5, 7]

                def lhs_of(dt):
                    return sel[dt % 2][:, dt // 2, :]
            else:
                zt0 = ztp.tile([128, NDT, 128], BF16, tag="z0f",
                               name=f"zt0_{rb}")
                nc.sync.dma_start(
                    out=zt0,
                    in_=entry[0:1024, :].rearrange("(t q) r -> q t r", q=128))
                zt1 = ztp.tile([128, NDT, 128], BF16, tag="z1f",
                               name=f"zt1_{rb}")
                nc.sync.dma_start(
                    out=zt1,
                    in_=entry[1024:2048, :].rearrange("(t q) r -> q t r", q=128))
                ztf = ztp.tile([128, NDT, 128], BF16, tag="zsf",
                               name=f"zt{rb}")
                nc.vector.tensor_scalar_mul(
                    out=ztf, in0=zt1, scalar1=bsel_sb[:, 1:2])
                nc.vector.scalar_tensor_tensor(
                    out=ztf, in0=zt0, scalar=bsel_sb[:, 0:1], in1=ztf,
                    op0=ALU.mult, op1=ALU.add,
                )
                dt_seq = list(range(NDT))

                def lhs_of(dt):
                    return ztf[:, dt, :]
            for half in range(2):
                psy = psp.tile([128, 512], F32, tag="p", name=f"py{rb}_{half}")
                for i, dt in enumerate(dt_seq):
                    nc.tensor.matmul(
                        psy,
                        lhsT=lhs_of(dt),
                        rhs=wo_sb[:, dt, half * 512:(half + 1) * 512],
                        start=(i == 0), stop=(i == NDT - 1),
                    )
                nc.vector.tensor_copy(
                    out=ystage[:, rb, half * 512:(half + 1) * 512], in_=psy)
            stats = lnp.tile([128, 2, 6], F32, tag="st", name=f"st{rb}")
            nc.vector.bn_stats(out=stats[:, 0, :], in_=ystage[:, rb, 0:512])
            nc.vector.bn_stats(out=stats[:, 1, :], in_=ystage[:, rb, 512:1024])
            mv = lnp.tile([128, 2], F32, tag="mv", name=f"mv{rb}")
            nc.vector.bn_aggr(out=mv, in_=stats)
            mv_tiles.append(mv)

        zn_dbg = None
        for rb in range(NRB):
            zn = ev.tile([128, HPC, 512], BF16, tag="zn", name=f"zn{rb}")
            if rb == NRB - 1:
                zn_dbg = zn
            for pair in range(2):
                zA = psz.tile([128, 512], F32, tag="z", name=f"z{rb}_{pair}a")
                zB = psz.tile([128, 512], F32, tag="z", name=f"z{rb}_{pair}b")
                first_v = (rb == 0 and pair == 0)
                s_cur = emit_scores(rb, pair, 0)
                if first_v:
                    emit_vproj(0)
                for kt in range(nkt):
                    if first_v and kt < len(kblocks) - 1:
                        # K proj for block kt+1 lands here: its kt tiles
                        # are first consumed 4 kt iterations later
                        emit_kproj(kt + 1)
                    if kt + 1 < nkt:
                        s_nxt = emit_scores(rb, pair, kt + 1)
                        if first_v:
                            emit_vproj(kt + 1)
                    pt = ptp.tile([128, 1024], BF16, tag="pt")
                    nc.scalar.activation(out=pt, in_=s_cur, func=AF.Exp)
                    nc.tensor.matmul(
                        zA[0:65, :], lhsT=v_res[:, kt, 2 * pair, :],
                        rhs=pt[:, 0:512],
                        start=(kt == 0), stop=(kt == nkt - 1),
                        skip_group_check=True,
                    )
                    nc.tensor.matmul(
                        zB[0:65, :], lhsT=v_res[:, kt, 2 * pair + 1, :],
                        rhs=pt[:, 512:1024],
                        start=(kt == 0), stop=(kt == nkt - 1),
                        skip_group_check=True,
                    )
                    if kt + 1 < nkt:
                        s_cur = s_nxt
                # stage raw z, then normalize: z = (z_raw - bv)*r + bv
                # (denominator d already includes the dummy key's +1)
                zr = ev.tile([128, 2, 512], BF16, tag="zr", name=f"zr{rb}{pair}")
                for hh, zX in ((0, zA), (1, zB)):
                    h = 2 * pair + hh
                    nc.vector.tensor_copy(out=zr[0:64, hh, :], in_=zX[0:64, :])
                    # custom DVE ops need a base-0 partition tile: stage the
                    # denominator row before reciprocal_approx_fast
                    dn = ev.tile([128, 512], F32, tag="dn", bufs=4,
                                 name=f"dn{rb}_{h}")
                    nc.vector.tensor_copy(out=dn[0:1, :], in_=zX[64:65, :])
                    rn = ev.tile([128, 512], F32, tag="rn", bufs=4,
                                 name=f"rn{rb}_{h}")
                    nc.vector.reciprocal_approx_fast(
                        out=rn[0:1, :], in_=dn[0:1, :])
                    rnb = ev.tile([128, 512], BF16, tag="rnb", bufs=4,
                                  name=f"rnb{rb}_{h}")
                    nc.vector.tensor_copy(out=rnb[0:1, :], in_=rn[0:1, :])
                    # row-broadcast on the tensor engine: ones^T @ r
                    rbc = psp.tile([128, 512], F32, tag="p", name=f"rbc{rb}_{h}")
                    nc.tensor.matmul(rbc[0:64, :], lhsT=ones_c, rhs=rnb[0:1, :],
                                     start=True, stop=True)
                    nc.vector.scalar_tensor_tensor(
                        out=zn[0:64, h, :], in0=zr[0:64, hh, :],
                        scalar=bvt_sb[:, h:h + 1],
                        in1=rbc[0:64, :], op0=ALU.subtract, op1=ALU.mult,
                    )
                    nc.vector.tensor_scalar_add(
                        out=zn[0:64, h, :], in0=zn[0:64, h, :],
                        scalar1=bvt_sb[:, h:h + 1],
                    )
                if rb in SPLIT_RB:
                    # ---- per-pair half AllToAll: z^T for 2 heads, dup
                    # per batch group; rb0's starts the serialized cc
                    # stream mid-block, rb3's halves the last transfer
                    a2a_in = dram.tile([1024, 128], BF16, tag=f"ai{pair}",
                                       name=f"ai{rb}_{pair}")
                    a2a_out = dram.tile([1024, 128], BF16, tag=f"ao{pair}",
                                        name=f"ao{rb}_{pair}")
                    for j in range(8):
                        nc.sync.dma_start(
                            out=a2a_in[128 * j:128 * (j + 1), :].rearrange(
                                "(h s) r -> s h r", h=2),
                            in_=zn[0:64, 2 * pair:2 * pair + 2,
                                   128 * (j % 4):128 * (j % 4 + 1)],
                        )
                    nc.gpsimd.collective_compute(
                        "AllToAll",
                        ALU.bypass,
                        replica_groups=[[0, 1, 2, 3, 4, 5, 6, 7]],
                        ins=[a2a_in[:, :].opt()],
                        outs=[a2a_out[:, :].opt()],
                    )
                    a2a_halves.append(a2a_out)
                    if pair == 1:
                        a2a_outs.append((a2a_halves[-2], a2a_halves[-1]))
                if pair == 0 and rb + 1 < NRB:
                    emit_qproj(0, rb + 1)
                    emit_qproj(1, rb + 1)

            if rb not in SPLIT_RB:
                # ---- full-rb AllToAll (lower fixed cost on the stream)
                a2a_in = dram.tile([2048, 128], BF16, tag="aif",
                                   name=f"aif{rb}")
                a2a_out = dram.tile([2048, 128], BF16, tag="aof",
                                    name=f"aof{rb}")
                for j in range(8):
                    nc.sync.dma_start(
                        out=a2a_in[256 * j:256 * (j + 1), :].rearrange(
                            "(h s) r -> s h r", h=HPC),
                        in_=zn[0:64, :, 128 * (j % 4):128 * (j % 4 + 1)],
                    )
                nc.gpsimd.collective_compute(
                    "AllToAll",
                    ALU.bypass,
                    replica_groups=[[0, 1, 2, 3, 4, 5, 6, 7]],
                    ins=[a2a_in[:, :].opt()],
                    outs=[a2a_out[:, :].opt()],
                )
                a2a_outs.append(a2a_out)
            if rb >= 1:
                emit_tail(rb - 1)
        emit_tail(NRB - 1)

        if debug:
            nc.sync.dma_start(out=dbg_kt[:, :, :], in_=kT_res[:, :, :])
            nc.sync.dma_start(out=dbg_qa[:, :, :], in_=qa_sb[:, :, :])
            nc.sync.dma_start(out=dbg_v[:, :, :, :], in_=v_res[:, :, :, :])
            nc.sync.dma_start(out=dbg_y[:, :, :], in_=ystage[:, :, :])
            nc.sync.dma_start(out=dbg_zn[:, :, :], in_=zn_dbg[:, :, :])
            nc.sync.dma_start(out=dbg_ao[:, :], in_=a2a_outs[3][0][:, :])

        # ---- LayerNorm finish (single ACT table switch to Sqrt) ----
        for rb in range(NRB):
            mv = mv_tiles[rb]
            std = lnp.tile([128, 1], F32, tag="sd", name=f"sd{rb}")
            nc.scalar.activation(
                out=std, in_=mv[:, 1:2], func=AF.Sqrt, bias=eps_sb[:, 0:1])
            rstd = lnp.tile([128, 1], F32, tag="rs", name=f"rs{rb}")
            nc.vector.reciprocal(out=rstd, in_=std)
            nmr = lnp.tile([128, 1], F32, tag="nm", name=f"nm{rb}")
            nc.vector.scalar_tensor_tensor(
                out=nmr, in0=mv[:, 0:1], scalar=-1.0, in1=rstd,
                op0=ALU.mult, op1=ALU.mult,
            )
            if ln_trivial:
                yv = lnp.tile([128, ED], F32, tag="y", name=f"y{rb}")
                nc.scalar.activation(
                    out=yv, in_=ystage[:, rb, :], func=AF.Identity,
                    scale=rstd[:, 0:1], bias=nmr[:, 0:1])
                nc.scalar.dma_start(out=out[rb * 128:(rb + 1) * 128, :], in_=yv)
            else:
                yb = lnp.tile([128, ED], BF16, tag="y", name=f"y{rb}")
                nc.scalar.activation(
                    out=yb, in_=ystage[:, rb, :], func=AF.Identity,
                    scale=rstd[:, 0:1], bias=nmr[:, 0:1])
                nc.vector.tensor_mul(out=yb, in0=yb, in1=g_bc)
                nc.vector.tensor_add(out=yb, in0=yb, in1=b_bc)
                nc.gpsimd.dma_start(out=out[rb * 128:(rb + 1) * 128, :], in_=yb)

    return nc


def prep_in_maps(query, key, value, attention_mask, pos_attn_score,
                 W_Q, b_Q, W_K, b_K, W_V, b_V, W_O, ln_gamma, ln_beta):
    import ml_dtypes
    f32 = np.float32
    bf16 = ml_dtypes.bfloat16

    q3 = np.asarray(query, f32)
    k3 = np.asarray(key, f32)
    v3 = np.asarray(value, f32)
    mask = np.asarray(attention_mask).astype(bool)
    pos = np.asarray(pos_attn_score, f32)

    idxs = [np.where(mask[b])[0] for b in range(B)]
    counts = [len(ix) for ix in idxs]
    # always at least one pad key (the dummy)
    skp = max(128, ((max(counts) + 1 + 127) // 128) * 128)
    nkt = skp // 128

    # per batch: compacted & padded keys/values/m-factors
    kc_b, vc_b, m_b = [], [], []
    for b in range(B):
        n = counts[b]
        kc = np.zeros((skp, D), f32)
        vc = np.zeros((skp, D), f32)
        mc = np.zeros((skp, H), f32)
        kc[:n] = k3[b][idxs[b]]
        vc[:n] = v3[b][idxs[b]]
        mc[:n] = np.exp(pos[b][idxs[b]])
        mc[skp - 1] = 1.0   # dummy key: weight exp(0)*1 = 1
        kc_b.append(kc)
        vc_b.append(vc)
        m_b.append(mc)

    def sb_img3(a2d):  # [D, cols] -> [128, NDT, cols] SBUF byte image
        return np.ascontiguousarray(
            a2d.reshape(NDT, 128, a2d.shape[1]).transpose(1, 0, 2)).astype(bf16)

    wqf = np.asarray(W_Q, f32).transpose(2, 1, 0)  # [D, H, HS]
    wkf = np.asarray(W_K, f32).transpose(2, 1, 0)
    wvf = np.asarray(W_V, f32).transpose(2, 1, 0)
    wof = np.asarray(W_O, f32).transpose(1, 2, 0).reshape(H * HS, ED)
    wo_img = sb_img3(wof)
    bqf = np.asarray(b_Q, f32)  # [H, HS]
    bkf = np.asarray(b_K, f32)
    bvf = np.asarray(b_V, f32)

    gamma = np.asarray(ln_gamma, f32)
    beta = np.asarray(ln_beta, f32)
    ln_trivial = bool(np.all(gamma == 1.0) and np.all(beta == 0.0))
    lngf = np.ascontiguousarray(gamma.reshape(1, ED)).astype(bf16)
    lnbf = np.ascontiguousarray(beta.reshape(1, ED)).astype(bf16)

    in_maps = []
    for c in range(NCORES):
        b, g = c // GROUP, c % GROUP
        heads = [4 * g + i for i in range(HPC)]
        wq_c = (wqf[:, heads, :] / 8.0).reshape(D, HPC * HS)
        wk_c = wkf[:, heads, :].reshape(D, HPC * HS)
        wv_c = wvf[:, heads, :].reshape(D, HPC * HS)
        bq_c = np.ascontiguousarray(
            (bqf[heads] / 8.0).reshape(2, 128).T)  # [128, 2] pair-packed
        bk_c = np.ascontiguousarray(bkf[heads].reshape(2, 128).T)
        bv_c = np.ascontiguousarray(bvf[heads].T)  # [64, 4]
        # dummy key column: solve wk_c^T x = -bk so its score is ~0
        bk_flat = bkf[heads].reshape(HPC * HS)
        cvec = np.linalg.lstsq(wk_c.T, -bk_flat, rcond=None)[0]
        kc = kc_b[b].copy()
        kc[skp - 1] = cvec
        xk_img = sb_img3(np.ascontiguousarray(kc.T))
        xv_img = sb_img3(np.ascontiguousarray(vc_b[b].T))
        # xq: [128, NRB, NDT, RBS] image
        xq_img = np.ascontiguousarray(
            q3[b].reshape(NRB, RBS, NDT, 128).transpose(3, 0, 2, 1)
        ).astype(bf16)
        m_c = np.zeros((128, nkt * HPC + KVER), f32)
        for kt in range(nkt):
            for hl in range(HPC):
                m_c[:, kt * HPC + hl] = m_b[b][kt * 128:(kt + 1) * 128,
                                               heads[hl]]
        bsel_c = np.zeros((128, 2), f32)
        bsel_c[:, b] = 1.0
        in_maps.append({
            "xq": xq_img, "xk": xk_img, "xv": xv_img,
            "wq": sb_img3(wq_c), "wk": sb_img3(wk_c), "wv": sb_img3(wv_c),
            "wo": wo_img,
            "bq": bq_c, "bk": bk_c, "bvt": bv_c, "mt": m_c, "bsel": bsel_c,
            **({} if ln_trivial else {"lng": lngf, "lnb": lnbf}),
        })
    return in_maps, skp, ln_trivial


def kernel(**inputs):
    global LAST_EXEC_NS
    in_maps, skp, ln_trivial = prep_in_maps(**inputs)
    key = (skp, ln_trivial)
    if key not in _CACHED:
        nc = _build(skp, ln_trivial)
        nc.finalize()
        _CACHED[key] = nc
    nc = _CACHED[key]

    trace = bool(os.environ.get("BASS_TRACE"))
    res = run_bass_kernel_spmd(nc, in_maps, core_ids=list(range(NCORES)),
                               trace=trace)
    LAST_EXEC_NS = res.exec_time_ns
    _CACHED["last_result"] = res

    out = np.empty((B, SQ, ED), np.float32)
    for c in range(NCORES):
        b, g = c // GROUP, c % GROUP
        o = res.results[c]["out"]  # [512, 1024]
        for rb in range(NRB):
            rows = slice(rb * RBS + g * 128, rb * RBS + (g + 1) * 128)
            out[b, rows] = o[rb * 128:(rb + 1) * 128]
    return out.reshape(B, SQ, ED)


# revision 38
# speedup vs baseline: 1.0425x; 1.0425x over previous
"""Distributed Trainium2 Bass kernel for the AttentionBlock problem.

Math (per batch b):
  q/k/v = x @ W + b ; scores = (q.k^T)/8 + pos[b,k,h], masked -> -inf,
  dummy col 0 ; pattern = softmax ; out = LayerNorm((pattern @ v) @ W_O)

Strategy (8 cores = 2 batches x 4 head-groups of 4 heads):
  * Host-side key compaction: masked keys are removed; key axis 2048 ->
    ~1046, padded to skp (mult of 128, always >= 1 pad).  Pad keys carry
    m=0 so they are exactly inert.  The LAST pad key is the softmax
    dummy: host solves W_K^T c = -b_K (lstsq) for its x-column so its
    projected k is ~0 -> score ~0 = DUMMY_SCORE, and m=1 -> it adds
    exactly +1 to the denominator via the 65th V column.  No device-side
    masking, biasing, or +1 ops anywhere.
  * Multiplicative softmax rewrite: exp(q.k/8 + pos) = exp(q.k/8)*m,
    m = exp(pos) host-computed; m scales V rows and the denominator
    column.  1/8 folded into W_Q/b_Q host-side.
  * All inputs are host-pre-swizzled into the exact SBUF byte layout so
    every load is a contiguous-run DMA (~line rate); loads are split
    across the sync/scalar/gpsimd queues and interleaved so K-proj ->
    first scores start within a few us.
  * Scores: two heads of a pair packed into one PE pass via row tiling
    (K=64 each, concurrent) into a 2-bank PSUM tile; one ACT exp call
    covers both heads (N=1024).  Scores are emitted one kt ahead of the
    z matmuls so the PE never waits on the ACT exp (ACT is the phase-2
    bottleneck at ~1us/kt).
  * z: per-head matmul with a 65th column of m in V, accumulating the
    softmax denominator for free.  bias b_V enters via
    z = (z_raw - b_V)/d + b_V.
  * Per 512-row block, per head-pair: a half-size 8-core AllToAll
    exchanges z^T (dup per batch group, receiver selects its batch via
    input-driven 0/1 scalars) so every core out-projects only its own
    128 rows.  Pair-level splitting starts the serialized cc stream
    mid-block and halves the last transfer.  A tiny warmup AllToAll is
    the first instruction of the kernel: it absorbs the ~70us cold-
    stream penalty + inter-core skew that otherwise hits the first real
    exchange.  Tail (out-proj) for block rb runs during block rb+1's
    attention; LayerNorm is deferred to the end (single ACT table
    switch), finished with one ACT Identity(scale=rstd, bias=-mu*rstd)
    per block.
"""

import os
from contextlib import ExitStack

import numpy as np

import concourse.bass as bass
import concourse.tile as tile
from concourse import bacc, mybir
from concourse.bass_utils import run_bass_kernel_spmd

B, SQ = 2, 2048
D = 1024
H, HS = 16, 64
ED = 1024
NCORES = 8
GROUP = 4          # cores per batch
HPC = 4            # heads per core
NRB = 4            # 512-row blocks per batch
RBS = 512
NDT = D // 128

F32 = mybir.dt.float32
BF16 = mybir.dt.bfloat16
FP8 = mybir.dt.float8e4
AF = mybir.ActivationFunctionType
ALU = mybir.AluOpType

LN_EPS = 1e-5
KVER = 11   # bump on every kernel revision: pads mt's shape so the HLO
           # (and thus the NEFF compile-cache key) is unique per version

LAST_EXEC_NS = None
_CACHED = {}


def _build(skp, ln_trivial, debug=False):
    nkt = skp // 128
    kblocks = [(s, min(512, skp - s)) for s in range(0, skp, 512)]

    nc = bacc.Bacc(None, target_bir_lowering=False)

    xq = nc.dram_tensor("xq", [128, NRB, NDT, RBS], BF16, kind="ExternalInput")
    xk = nc.dram_tensor("xk", [128, NDT, skp], BF16, kind="ExternalInput")
    xv = nc.dram_tensor("xv", [128, NDT, skp], BF16, kind="ExternalInput")
    wq = nc.dram_tensor("wq", [128, NDT, HPC * HS], BF16, kind="ExternalInput")
    wk = nc.dram_tensor("wk", [128, NDT, HPC * HS], BF16, kind="ExternalInput")
    wv = nc.dram_tensor("wv", [128, NDT, HPC * HS], BF16, kind="ExternalInput")
    wo = nc.dram_tensor("wo", [128, NDT, ED], BF16, kind="ExternalInput")
    bq = nc.dram_tensor("bq", [128, 2], F32, kind="ExternalInput")
    bk = nc.dram_tensor("bk", [128, 2], F32, kind="ExternalInput")
    bvt = nc.dram_tensor("bvt", [64, HPC], F32, kind="ExternalInput")
    bsel = nc.dram_tensor("bsel", [128, 2], F32, kind="ExternalInput")
    mt = nc.dram_tensor("mt", [128, nkt * HPC + KVER], F32, kind="ExternalInput")
    if not ln_trivial:
        lng = nc.dram_tensor("lng", [1, ED], BF16, kind="ExternalInput")
        lnb = nc.dram_tensor("lnb", [1, ED], BF16, kind="ExternalInput")
    out = nc.dram_tensor("out", [NRB * 128, ED], F32, kind="ExternalOutput")
    if debug:
        dbg_kt = nc.dram_tensor("dbg_kt", [128, 2, skp], BF16,
                                kind="ExternalOutput")
        dbg_qa = nc.dram_tensor("dbg_qa", [128, 2, SQ], BF16,
                                kind="ExternalOutput")
        dbg_v = nc.dram_tensor("dbg_v", [128, nkt, HPC, 65], BF16,
                               kind="ExternalOutput")
        dbg_y = nc.dram_tensor("dbg_y", [128, NRB, ED], BF16,
                               kind="ExternalOutput")
        dbg_zn = nc.dram_tensor("dbg_zn", [128, HPC, 512], BF16,
                                kind="ExternalOutput")
        dbg_ao = nc.dram_tensor("dbg_ao", [1024, 128], BF16,
                                kind="ExternalOutput")

    with tile.TileContext(nc) as tc, ExitStack() as ctx:
        consts = ctx.enter_context(tc.tile_pool(name="consts", bufs=1))
        res = ctx.enter_context(tc.tile_pool(name="res", bufs=1))
        dram = ctx.enter_context(tc.tile_pool(name="dram", bufs=8, space="DRAM"))
        pss = ctx.enter_context(tc.tile_pool(name="pss", bufs=2, space="PSUM"))
        psz = ctx.enter_context(tc.tile_pool(name="psz", bufs=2, space="PSUM"))
        psp = ctx.enter_context(tc.tile_pool(name="psp", bufs=2, space="PSUM"))
        ptp = ctx.enter_context(tc.tile_pool(name="ptp", bufs=3))
        ev = ctx.enter_context(tc.tile_pool(name="ev", bufs=2))
        ztp = ctx.enter_context(tc.tile_pool(name="ztp", bufs=2))
        lnp = ctx.enter_context(tc.tile_pool(name="lnp", bufs=4))

        # ---- constants (scalar queue, tiny, first) ----
        bq_sb = consts.tile([128, 2], F32)
        nc.scalar.dma_start(out=bq_sb, in_=bq[:, :])
        bk_sb = consts.tile([128, 2], F32)
        nc.scalar.dma_start(out=bk_sb, in_=bk[:, :])
        m_sb = consts.tile([128, nkt, HPC], F32)
        nc.scalar.dma_start(out=m_sb, in_=mt[:, 0:nkt * HPC].rearrange(
            "p (t h) -> p t h", t=nkt))
        bvt_sb = consts.tile([64, HPC], F32)
        nc.scalar.dma_start(out=bvt_sb, in_=bvt[:, :])
        bsel_sb = consts.tile([128, 2], F32)
        nc.scalar.dma_start(out=bsel_sb, in_=bsel[:, :])
        eps_sb = consts.tile([128, 1], F32)
        nc.vector.memset(eps_sb, LN_EPS)
        ones_c = consts.tile([1, 64], BF16)
        nc.vector.memset(ones_c, 1.0)
        # PE warm-up spin: ~4us of dummy matmuls while input DMAs stream
        # flips the HAM clock gate to 2.4GHz before the real projections;
        # the dummy exp preloads the ACT exp table set (~2.7us) at t=0
        wrm = consts.tile([128, 16], BF16)
        nc.vector.memset(wrm, 0.0)
        wex = consts.tile([128, 1], F32)
        nc.scalar.activation(out=wex, in_=eps_sb, func=AF.Exp)
        ps_w = psp.tile([128, 512], F32, tag="p", name="warm")
        for _ in range(64):
            nc.tensor.matmul(ps_w[0:16, 0:16], lhsT=wrm, rhs=wrm,
                             start=True, stop=True)

        # ---- persistent tiles ----
        kT_res = res.tile([128, 2, skp], BF16)       # [hs(pair-packed), pair, key]
        qa_sb = res.tile([128, 2, SQ], BF16)         # [hs(pair-packed), pair, row]
        v_res = res.tile([128, nkt, HPC, 128], BF16)
        # [key, kt, head, hs|m|pad]: 128 cols (not 65) so the z-matmul
        # LDWEIGHTS hits the FWL fast path; pad cols are zeroed once and
        # produce PSUM rows 65..127 that nothing reads
        nc.vector.memset(v_res, 0.0)
        wo_sb = res.tile([128, NDT, ED], BF16)
        ystage = res.tile([128, NRB, ED], BF16)
        xq_sb = res.tile([128, NRB, NDT, RBS], BF16)
        xk_sb = res.tile([128, NDT, skp], BF16)
        xv_sb = res.tile([128, NDT, skp], BF16)
        wq_sb = res.tile([128, NDT, HPC * HS], BF16)
        wk_sb = res.tile([128, NDT, HPC * HS], BF16)
        wv_sb = res.tile([128, NDT, HPC * HS], BF16)

        # ---- input DMAs ----
        # SWDGE (gpsimd) sprays across all 16 SDMA engines (~430 GB/s
        # measured) while HWDGE (sync/scalar) moves ~100-150 GB/s: the
        # whole critical path rides gpsimd, ordered so K-proj -> Q-proj
        # -> first scores unblock ASAP.  sync gets the late q blocks,
        # scalar the tail-only wo.
        # warmup collective FIRST (before any DMA): the first AllToAll on
        # a cold cc stream runs 3-10x slower and absorbs inter-core start
        # skew; firing it at t=0 takes both off the critical path
        cw_in = dram.tile([128, 128], BF16, tag="cw", name="cwi")
        cw_out = dram.tile([128, 128], BF16, tag="cw2", name="cwo")
        nc.gpsimd.collective_compute(
            "AllToAll",
            ALU.bypass,
            replica_groups=[[0, 1, 2, 3, 4, 5, 6, 7]],
            ins=[cw_in[:, :].opt()],
            outs=[cw_out[:, :].opt()],
        )
        nc.gpsimd.dma_start(out=wk_sb, in_=wk[:, :, :])
        ks0, kw0 = kblocks[0]
        nc.gpsimd.dma_start(
            out=xk_sb[:, :, ks0:ks0 + kw0], in_=xk[:, :, ks0:ks0 + kw0])
        nc.gpsimd.dma_start(out=wq_sb, in_=wq[:, :, :])
        nc.gpsimd.dma_start(out=xq_sb[:, 0], in_=xq[:, 0])
        nc.gpsimd.dma_start(out=wv_sb, in_=wv[:, :, :])
        nc.gpsimd.dma_start(out=xv_sb[:, :, 0:128], in_=xv[:, :, 0:128])
        for (ks, kw) in kblocks[1:]:
            nc.gpsimd.dma_start(
                out=xk_sb[:, :, ks:ks + kw], in_=xk[:, :, ks:ks + kw])
        for ks in range(128, skp, 512):
            kw = min(512, skp - ks)
            nc.gpsimd.dma_start(
                out=xv_sb[:, :, ks:ks + kw], in_=xv[:, :, ks:ks + kw])
        if not ln_trivial:
            g_bc = consts.tile([128, ED], BF16)
            b_bc = consts.tile([128, ED], BF16)
            nc.gpsimd.dma_start(out=g_bc, in_=lng[:, :].to_broadcast([128, ED]))
            nc.gpsimd.dma_start(out=b_bc, in_=lnb[:, :].to_broadcast([128, ED]))
        # sync: q block 1 then (later) a2a staging + zt loads
        nc.sync.dma_start(out=xq_sb[:, 1], in_=xq[:, 1])
        # scalar: late q blocks + wo (needed from rb2 / tail0 on)
        nc.scalar.dma_start(out=xq_sb[:, 2], in_=xq[:, 2])
        nc.scalar.dma_start(out=xq_sb[:, 3], in_=xq[:, 3])
        nc.scalar.dma_start(out=wo_sb, in_=wo[:, :, :])

        def emit_qproj(pair, qb):
            ps = psp.tile([128, 512], F32, tag="p", name=f"pq{pair}_{qb}")
            for dt in range(NDT):
                nc.tensor.matmul(
                    ps,
                    lhsT=wq_sb[:, dt, pair * 128:(pair + 1) * 128],
                    rhs=xq_sb[:, qb, dt, :],
                    start=(dt == 0), stop=(dt == NDT - 1),
                )
            nc.vector.tensor_scalar_add(
                out=qa_sb[:, pair, qb * RBS:(qb + 1) * RBS], in0=ps,
                scalar1=bq_sb[:, pair:pair + 1],
            )

        def emit_kproj(bi):
            ks, kw = kblocks[bi]
            for pair in range(2):
                ps = psp.tile([128, 512], F32, tag="p", name=f"pk{pair}_{ks}")
                for dt in range(NDT):
                    nc.tensor.matmul(
                        ps[:, 0:kw],
                        lhsT=wk_sb[:, dt, pair * 128:(pair + 1) * 128],
                        rhs=xk_sb[:, dt, ks:ks + kw],
                        start=(dt == 0), stop=(dt == NDT - 1),
                    )
                nc.vector.tensor_scalar_add(
                    out=kT_res[:, pair, ks:ks + kw], in0=ps[:, 0:kw],
                    scalar1=bk_sb[:, pair:pair + 1],
                )

        # ---- phase 1: K proj block 0 + first Q block; K blocks 1+ are
        # interleaved into rb0/pair0 (their kt tiles are consumed later)
        emit_kproj(0)
        emit_qproj(0, 0)
        emit_qproj(1, 0)

        def emit_vproj(kt):
            # V projection for one key tile, scaled by m; 65th col = m
            ps = psp.tile([128, 512], F32, tag="p", name=f"pv{kt}")
            for dt in range(NDT):
                nc.tensor.matmul(
                    ps[:, 0:HPC * HS],
                    lhsT=xv_sb[:, dt, kt * 128:(kt + 1) * 128],
                    rhs=wv_sb[:, dt, :],
                    start=(dt == 0), stop=(dt == NDT - 1),
                )
            for hl in range(HPC):
                nc.vector.tensor_scalar_mul(
                    out=v_res[:, kt, hl, 0:64],
                    in0=ps[:, hl * 64:(hl + 1) * 64],
                    scalar1=m_sb[:, kt, hl:hl + 1],
                )
            nc.vector.tensor_copy(out=v_res[:, kt, :, 64], in_=m_sb[:, kt, :])

        # ---- phase 2 ----
        a2a_outs = []
        a2a_halves = []
        mv_tiles = []
        SPLIT_RB = {0, NRB - 1}

        def emit_scores(rb, pair, kt):
            rs = slice(rb * RBS, (rb + 1) * RBS)
            s2 = pss.tile([128, 1024], F32, tag="s", name=f"s{rb}{pair}{kt}")
            nc.tensor.matmul(
                s2[:, 0:512],
                lhsT=kT_res[0:64, pair, kt * 128:(kt + 1) * 128],
                rhs=qa_sb[0:64, pair, rs],
                start=True, stop=True,
            )
            nc.tensor.matmul(
                s2[:, 512:1024],
                lhsT=kT_res[64:128, pair, kt * 128:(kt + 1) * 128],
                rhs=qa_sb[64:128, pair, rs],
                start=True, stop=True,
            )
            return s2

        def emit_tail(rb):
            # out-projection of this core's 128 rows for block rb; load
            # both batch halves, select mine via input-driven 0/1 scalar.
            # Split-rb form: dt tile (2j + p) <-> sel[p][:, j]; MMs run
            # pair0's dts first so they start before pair1's exchange
            # lands.  Full-rb form: one [128, 8, 128] zt, natural order.
            entry = a2a_outs[rb]
            split = isinstance(entry, tuple)
            if split:
                sel = []
                for p in range(2):
                    ao = entry[p]
                    zt0 = ztp.tile([128, 4, 128], BF16, tag=f"z0{p}",
                                   name=f"zt0_{rb}{p}")
                    nc.sync.dma_start(
                        out=zt0,
                        in_=ao[0:512, :].rearrange("(t q) r -> q t r", q=128))
                    zt1 = ztp.tile([128, 4, 128], BF16, tag=f"z1{p}",
                                   name=f"zt1_{rb}{p}")
                    nc.sync.dma_start(
                        out=zt1,
                        in_=ao[512:1024, :].rearrange("(t q) r -> q t r", q=128))
                    zt = ztp.tile([128, 4, 128], BF16, tag=f"zs{p}",
                                  name=f"zt{rb}{p}")
                    nc.vector.tensor_scalar_mul(
                        out=zt, in0=zt1, scalar1=bsel_sb[:, 1:2])
                    nc.vector.scalar_tensor_tensor(
                        out=zt, in0=zt0, scalar=bsel_sb[:, 0:1], in1=zt,
                        op0=ALU.mult, op1=ALU.add,
                    )
                    sel.append(zt)
                dt_seq = [0, 2, 4, 6, 1, 3, 5, 7]

                def lhs_of(dt):
                    return sel[dt % 2][:, dt // 2, :]
            else:
                zt0 = ztp.tile([128, NDT, 128], BF16, tag="z0f",
                               name=f"zt0_{rb}")
                nc.sync.dma_start(
                    out=zt0,
                    in_=entry[0:1024, :].rearrange("(t q) r -> q t r", q=128))
                zt1 = ztp.tile([128, NDT, 128], BF16, tag="z1f",
                               name=f"zt1_{rb}")
                nc.sync.dma_start(
                    out=zt1,
                    in_=entry[1024:2048, :].rearrange("(t q) r -> q t r", q=128))
                ztf = ztp.tile([128, NDT, 128], BF16, tag="zsf",
                               name=f"zt{rb}")
                nc.vector.tensor_scalar_mul(
                    out=ztf, in0=zt1, scalar1=bsel_sb[:, 1:2])
                nc.vector.scalar_tensor_tensor(
                    out=ztf, in0=zt0, scalar=bsel_sb[:, 0:1], in1=ztf,
                    op0=ALU.mult, op1=ALU.add,
                )
                dt_seq = list(range(NDT))

                def lhs_of(dt):
                    return ztf[:, dt, :]
            for half in range(2):
                psy = psp.tile([128, 512], F32, tag="p", name=f"py{rb}_{half}")
                for i, dt in enumerate(dt_seq):
                    nc.tensor.matmul(
                        psy,
                        lhsT=lhs_of(dt),
                        rhs=wo_sb[:, dt, half * 512:(half + 1) * 512],
                        start=(i == 0), stop=(i == NDT - 1),
                    )
                nc.vector.tensor_copy(
                    out=ystage[:, rb, half * 512:(half + 1) * 512], in_=psy)
            stats = lnp.tile([128, 2, 6], F32, tag="st", name=f"st{rb}")
            nc.vector.bn_stats(out=stats[:, 0, :], in_=ystage[:, rb, 0:512])
            nc.vector.bn_stats(out=stats[:, 1, :], in_=ystage[:, rb, 512:1024])
            mv = lnp.tile([128, 2], F32, tag="mv", name=f"mv{rb}")
            nc.vector.bn_aggr(out=mv, in_=stats)
            mv_tiles.append(mv)

        zn_dbg = None
        for rb in range(NRB):
            zn = ev.tile([128, HPC, 512], BF16, tag="zn", name=f"zn{rb}")
            if rb == NRB - 1:
                zn_dbg = zn
            for pair in range(2):
                zA = psz.tile([128, 512], F32, tag="z", name=f"z{rb}_{pair}a")
                zB = psz.tile([128, 512], F32, tag="z", name=f"z{rb}_{pair}b")
                first_v = (rb == 0 and pair == 0)
                s_cur = emit_scores(rb, pair, 0)
                if first_v:
                    emit_vproj(0)
                for kt in range(nkt):
                    if first_v and kt < len(kblocks) - 1:
                        # K proj for block kt+1 lands here: its kt tiles
                        # are first consumed 4 kt iterations later
                        emit_kproj(kt + 1)
                    if kt + 1 < nkt:
                        s_nxt = emit_scores(rb, pair, kt + 1)
                        if first_v:
                            emit_vproj(kt + 1)
                    pt = ptp.tile([128, 1024], BF16, tag="pt")
                    nc.scalar.activation(out=pt, in_=s_cur, func=AF.Exp)
                    nc.tensor.matmul(
                        zA[:, :], lhsT=v_res[:, kt, 2 * pair, :],
                        rhs=pt[:, 0:512],
                        start=(kt == 0), stop=(kt == nkt - 1),
                        skip_group_check=True,
                    )
                    nc.tensor.matmul(
                        zB[:, :], lhsT=v_res[:, kt, 2 * pair + 1, :],
                        rhs=pt[:, 512:1024],
                        start=(kt == 0), stop=(kt == nkt - 1),
                        skip_group_check=True,
                    )
                    if kt + 1 < nkt:
                        s_cur = s_nxt
                # stage raw z, then normalize: z = (z_raw - bv)*r + bv
                # (denominator d already includes the dummy key's +1)
                zr = ev.tile([128, 2, 512], BF16, tag="zr", name=f"zr{rb}{pair}")
                for hh, zX in ((0, zA), (1, zB)):
                    h = 2 * pair + hh
                    nc.vector.tensor_copy(out=zr[0:64, hh, :], in_=zX[0:64, :])
                    # custom DVE ops need a base-0 partition tile: stage the
                    # denominator row before reciprocal_approx_fast
                    dn = ev.tile([128, 512], F32, tag="dn", bufs=4,
                                 name=f"dn{rb}_{h}")
                    nc.vector.tensor_copy(out=dn[0:1, :], in_=zX[64:65, :])
                    rn = ev.tile([128, 512], F32, tag="rn", bufs=4,
                                 name=f"rn{rb}_{h}")
                    nc.vector.reciprocal_approx_fast(
                        out=rn[0:1, :], in_=dn[0:1, :])
                    rnb = ev.tile([128, 512], BF16, tag="rnb", bufs=4,
                                  name=f"rnb{rb}_{h}")
                    nc.vector.tensor_copy(out=rnb[0:1, :], in_=rn[0:1, :])
                    # row-broadcast on the tensor engine: ones^T @ r
                    rbc = psp.tile([128, 512], F32, tag="p", name=f"rbc{rb}_{h}")
                    nc.tensor.matmul(rbc[0:64, :], lhsT=ones_c, rhs=rnb[0:1, :],
                                     start=True, stop=True)
                    nc.vector.scalar_tensor_tensor(
                        out=zn[0:64, h, :], in0=zr[0:64, hh, :],
                        scalar=bvt_sb[:, h:h + 1],
                        in1=rbc[0:64, :], op0=ALU.subtract, op1=ALU.mult,
                    )
                    nc.vector.tensor_scalar_add(
                        out=zn[0:64, h, :], in0=zn[0:64, h, :],
                        scalar1=bvt_sb[:, h:h + 1],
                    )
                if rb in SPLIT_RB:
                    # ---- per-pair half AllToAll: z^T for 2 heads, dup
                    # per batch group; rb0's starts the serialized cc
                    # stream mid-block, rb3's halves the last transfer
                    a2a_in = dram.tile([1024, 128], BF16, tag=f"ai{pair}",
                                       name=f"ai{rb}_{pair}")
                    a2a_out = dram.tile([1024, 128], BF16, tag=f"ao{pair}",
                                        name=f"ao{rb}_{pair}")
                    for j in range(8):
                        nc.sync.dma_start(
                            out=a2a_in[128 * j:128 * (j + 1), :].rearrange(
                                "(h s) r -> s h r", h=2),
                            in_=zn[0:64, 2 * pair:2 * pair + 2,
                                   128 * (j % 4):128 * (j % 4 + 1)],
                        )
                    nc.gpsimd.collective_compute(
                        "AllToAll",
                        ALU.bypass,
                        replica_groups=[[0, 1, 2, 3, 4, 5, 6, 7]],
                        ins=[a2a_in[:, :].opt()],
                        outs=[a2a_out[:, :].opt()],
                    )
                    a2a_halves.append(a2a_out)
                    if pair == 1:
                        a2a_outs.append((a2a_halves[-2], a2a_halves[-1]))
                if pair == 0 and rb + 1 < NRB:
                    emit_qproj(0, rb + 1)
                    emit_qproj(1, rb + 1)

            if rb not in SPLIT_RB:
                # ---- full-rb AllToAll (lower fixed cost on the stream)
                a2a_in = dram.tile([2048, 128], BF16, tag="aif",
                                   name=f"aif{rb}")
                a2a_out = dram.tile([2048, 128], BF16, tag="aof",
                                    name=f"aof{rb}")
                for j in range(8):
                    nc.sync.dma_start(
                        out=a2a_in[256 * j:256 * (j + 1), :].rearrange(
                            "(h s) r -> s h r", h=HPC),
                        in_=zn[0:64, :, 128 * (j % 4):128 * (j % 4 + 1)],
                    )
                nc.gpsimd.collective_compute(
                    "AllToAll",
                    ALU.bypass,
                    replica_groups=[[0, 1, 2, 3, 4, 5, 6, 7]],
                    ins=[a2a_in[:, :].opt()],
                    outs=[a2a_out[:, :].opt()],
                )
                a2a_outs.append(a2a_out)
            if rb >= 1:
                emit_tail(rb - 1)
        emit_tail(NRB - 1)

        if debug:
            nc.sync.dma_start(out=dbg_kt[:, :, :], in_=kT_res[:, :, :])
            nc.sync.dma_start(out=dbg_qa[:, :, :], in_=qa_sb[:, :, :])
            nc.sync.dma_start(out=dbg_v[:, :, :, :], in_=v_res[:, :, :, 0:65])
            nc.sync.dma_start(out=dbg_y[:, :, :], in_=ystage[:, :, :])
            nc.sync.dma_start(out=dbg_zn[:, :, :], in_=zn_dbg[:, :, :])
            nc.sync.dma_start(out=dbg_ao[:, :], in_=a2a_outs[3][0][:, :])

        # ---- LayerNorm finish (single ACT table switch to Sqrt) ----
        for rb in range(NRB):
            mv = mv_tiles[rb]
            std = lnp.tile([128, 1], F32, tag="sd", name=f"sd{rb}")
            nc.scalar.activation(
                out=std, in_=mv[:, 1:2], func=AF.Sqrt, bias=eps_sb[:, 0:1])
            rstd = lnp.tile([128, 1], F32, tag="rs", name=f"rs{rb}")
            nc.vector.reciprocal(out=rstd, in_=std)
            nmr = lnp.tile([128, 1], F32, tag="nm", name=f"nm{rb}")
            nc.vector.scalar_tensor_tensor(
                out=nmr, in0=mv[:, 0:1], scalar=-1.0, in1=rstd,
                op0=ALU.mult, op1=ALU.mult,
            )
            if ln_trivial:
                yv = lnp.tile([128, ED], F32, tag="y", name=f"y{rb}")
                nc.scalar.activation(
                    out=yv, in_=ystage[:, rb, :], func=AF.Identity,
                    scale=rstd[:, 0:1], bias=nmr[:, 0:1])
                nc.scalar.dma_start(out=out[rb * 128:(rb + 1) * 128, :], in_=yv)
            else:
                yb = lnp.tile([128, ED], BF16, tag="y", name=f"y{rb}")
                nc.scalar.activation(
                    out=yb, in_=ystage[:, rb, :], func=AF.Identity,
                    scale=rstd[:, 0:1], bias=nmr[:, 0:1])
                nc.vector.tensor_mul(out=yb, in0=yb, in1=g_bc)
                nc.vector.tensor_add(out=yb, in0=yb, in1=b_bc)
                nc.gpsimd.dma_start(out=out[rb * 128:(rb + 1) * 128, :], in_=yb)

    return nc


def prep_in_maps(query, key, value, attention_mask, pos_attn_score,
                 W_Q, b_Q, W_K, b_K, W_V, b_V, W_O, ln_gamma, ln_beta):
    import ml_dtypes
    f32 = np.float32
    bf16 = ml_dtypes.bfloat16

    q3 = np.asarray(query, f32)
    k3 = np.asarray(key, f32)
    v3 = np.asarray(value, f32)
    mask = np.asarray(attention_mask).astype(bool)
    pos = np.asarray(pos_attn_score, f32)

    idxs = [np.where(mask[b])[0] for b in range(B)]
    counts = [len(ix) for ix in idxs]
    # always at least one pad key (the dummy)
    skp = max(128, ((max(counts) + 1 + 127) // 128) * 128)
    nkt = skp // 128

    # per batch: compacted & padded keys/values/m-factors
    kc_b, vc_b, m_b = [], [], []
    for b in range(B):
        n = counts[b]
        kc = np.zeros((skp, D), f32)
        vc = np.zeros((skp, D), f32)
        mc = np.zeros((skp, H), f32)
        kc[:n] = k3[b][idxs[b]]
        vc[:n] = v3[b][idxs[b]]
        mc[:n] = np.exp(pos[b][idxs[b]])
        mc[skp - 1] = 1.0   # dummy key: weight exp(0)*1 = 1
        kc_b.append(kc)
        vc_b.append(vc)
        m_b.append(mc)

    def sb_img3(a2d):  # [D, cols] -> [128, NDT, cols] SBUF byte image
        return np.ascontiguousarray(
            a2d.reshape(NDT, 128, a2d.shape[1]).transpose(1, 0, 2)).astype(bf16)

    wqf = np.asarray(W_Q, f32).transpose(2, 1, 0)  # [D, H, HS]
    wkf = np.asarray(W_K, f32).transpose(2, 1, 0)
    wvf = np.asarray(W_V, f32).transpose(2, 1, 0)
    wof = np.asarray(W_O, f32).transpose(1, 2, 0).reshape(H * HS, ED)
    wo_img = sb_img3(wof)
    bqf = np.asarray(b_Q, f32)  # [H, HS]
    bkf = np.asarray(b_K, f32)
    bvf = np.asarray(b_V, f32)

    gamma = np.asarray(ln_gamma, f32)
    beta = np.asarray(ln_beta, f32)
    ln_trivial = bool(np.all(gamma == 1.0) and np.all(beta == 0.0))
    lngf = np.ascontiguousarray(gamma.reshape(1, ED)).astype(bf16)
    lnbf = np.ascontiguousarray(beta.reshape(1, ED)).astype(bf16)

    in_maps = []
    for c in range(NCORES):
        b, g = c // GROUP, c % GROUP
        heads = [4 * g + i for i in range(HPC)]
        wq_c = (wqf[:, heads, :] / 8.0).reshape(D, HPC * HS)
        wk_c = wkf[:, heads, :].reshape(D, HPC * HS)
        wv_c = wvf[:, heads, :].reshape(D, HPC * HS)
        bq_c = np.ascontiguousarray(
            (bqf[heads] / 8.0).reshape(2, 128).T)  # [128, 2] pair-packed
        bk_c = np.ascontiguousarray(bkf[heads].reshape(2, 128).T)
        bv_c = np.ascontiguousarray(bvf[heads].T)  # [64, 4]
        # dummy key column: solve wk_c^T x = -bk so its score is ~0
        bk_flat = bkf[heads].reshape(HPC * HS)
        cvec = np.linalg.lstsq(wk_c.T, -bk_flat, rcond=None)[0]
        kc = kc_b[b].copy()
        kc[skp - 1] = cvec
        xk_img = sb_img3(np.ascontiguousarray(kc.T))
        xv_img = sb_img3(np.ascontiguousarray(vc_b[b].T))
        # xq: [128, NRB, NDT, RBS] image
        xq_img = np.ascontiguousarray(
            q3[b].reshape(NRB, RBS, NDT, 128).transpose(3, 0, 2, 1)
        ).astype(bf16)
        m_c = np.zeros((128, nkt * HPC + KVER), f32)
        for kt in range(nkt):
            for hl in range(HPC):
                m_c[:, kt * HPC + hl] = m_b[b][kt * 128:(kt + 1) * 128,
                                               heads[hl]]
        bsel_c = np.zeros((128, 2), f32)
        bsel_c[:, b] = 1.0
        in_maps.append({
            "xq": xq_img, "xk": xk_img, "xv": xv_img,
            "wq": sb_img3(wq_c), "wk": sb_img3(wk_c), "wv": sb_img3(wv_c),
            "wo": wo_img,
            "bq": bq_c, "bk": bk_c, "bvt": bv_c, "mt": m_c, "bsel": bsel_c,
            **({} if ln_trivial else {"lng": lngf, "lnb": lnbf}),
        })
    return in_maps, skp, ln_trivial


def kernel(**inputs):
    global LAST_EXEC_NS
    in_maps, skp, ln_trivial = prep_in_maps(**inputs)
    key = (skp, ln_trivial)
    if key not in _CACHED:
        nc = _build(skp, ln_trivial)
        nc.finalize()
        _CACHED[key] = nc
    nc = _CACHED[key]

    trace = bool(os.environ.get("BASS_TRACE"))
    res = run_bass_kernel_spmd(nc, in_maps, core_ids=list(range(NCORES)),
                               trace=trace)
    LAST_EXEC_NS = res.exec_time_ns
    _CACHED["last_result"] = res

    out = np.empty((B, SQ, ED), np.float32)
    for c in range(NCORES):
        b, g = c // GROUP, c % GROUP
        o = res.results[c]["out"]  # [512, 1024]
        for rb in range(NRB):
            rows = slice(rb * RBS + g * 128, rb * RBS + (g + 1) * 128)
            out[b, rows] = o[rb * 128:(rb + 1) * 128]
    return out.reshape(B, SQ, ED)
